# revision 21
# baseline (speedup 1.0000x reference)
"""Bahdanau-attention GRU decoder on 8 trn2 NeuronCores (Bass/Tile).

Sharding: vocab (32000) split 8x4000 (padded 4096) per core for the two big
GEMMs (logits = tm @ W_o, yemb = probs @ Ey); attention batch-sharded (8 rows
per core); GRU gate features sharded (128 h per core); deep-output replicated.
Per step: AllGather(ctx), AllGather(s_new^T chunk), AllReduce(yemb partial +
softmax-Z partial).  All matmul operands bf16, accumulation f32.
"""
import sys, os
sys.path.insert(0, '/opt/trn_rl_repo')

import numpy as np
import ml_dtypes

os.environ.setdefault("JAX_COMPILATION_CACHE_DIR", "/root/.jax_cache")
os.environ.setdefault("JAX_PERSISTENT_CACHE_MIN_ENTRY_SIZE_BYTES", "-1")
os.environ.setdefault("JAX_PERSISTENT_CACHE_MIN_COMPILE_TIME_SECS", "0")

BF16 = ml_dtypes.bfloat16
NCORES = 8
B, S, I, C, H, D, M, KY = 64, 64, 512, 512, 1024, 512, 512, 32000
KV = KY // NCORES          # 4000 real vocab cols per core
KVP = 4096                 # padded
BL = B // NCORES           # 8 local batch rows


def _kmaj(a, nk):
    """[nk*128, N] -> [128, nk, N] with (p, kc, n) = a[kc*128+p, n]."""
    K, N = a.shape
    assert K == nk * 128
    return np.ascontiguousarray(a.reshape(nk, 128, N).transpose(1, 0, 2))


def _build_in_maps(input_seq, Ey_t, W, U, b, v, W_ih, W_hh, b_ih, b_hh,
                   U_o, V_o, C_o, W_o):
    f32 = np.float32
    input_seq = np.asarray(input_seq, f32)
    U_h = (input_seq.reshape(B * S, I).astype(f32) @ np.asarray(U, f32)
           + np.asarray(b, f32)).reshape(B, S, C)
    W_ihT = np.asarray(W_ih, f32).T     # [C, 3H]
    W_hhT = np.asarray(W_hh, f32).T     # [H, 3H]
    bsum = np.asarray(b_ih, f32) + np.asarray(b_hh, f32)
    DW = np.vstack([np.asarray(U_o, f32), np.asarray(V_o, f32),
                    np.asarray(C_o, f32)])          # [2H, H]
    vatt = np.ascontiguousarray(np.asarray(v, f32).reshape(4, 128).T).astype(BF16)
    id64b = np.eye(64, dtype=BF16)
    id64f = np.eye(64, dtype=f32)
    W_o = np.asarray(W_o, f32)
    Ey_t = np.asarray(Ey_t, f32)

    in_maps = []
    for r in range(NCORES):
        hc = slice(128 * r, 128 * r + 128)
        gcols = np.r_[128 * r:128 * r + 128, 1024 + 128 * r:1024 + 128 * r + 128]
        ncols = np.arange(2048 + 128 * r, 2048 + 128 * r + 128)
        wrz = np.concatenate([_kmaj(W_ihT[:, gcols], 4),
                              _kmaj(W_hhT[:, gcols], 8)], axis=1)
        win = _kmaj(W_ihT[:, ncols], 4)
        whn = _kmaj(W_hhT[:, ncols], 8)
        wo = np.zeros((512, KVP), f32)
        wo[:, :KV] = W_o[:, KV * r:KV * r + KV]
        ey = np.zeros((KVP, 520), f32)
        ey[:KV, :512] = Ey_t[KV * r:KV * r + KV]
        ey[:KV, 512] = 1.0
        # chunk vc / array-col m holds vocab col m*32+vc so the output DMA
        # (partition-major) lands at linear offset p*32+vc
        perm = (np.arange(128)[None, :] * 32 + np.arange(32)[:, None]).reshape(-1)
        wo = wo[:, perm]
        ey = ey[perm, :]
        uh = U_h[BL * r:BL * r + BL].transpose(2, 0, 1)       # [C, BL, S]
        uh = uh.reshape(4, 128, BL, S).transpose(1, 0, 2, 3)  # [128, 4, BL, S]
        xl = input_seq[BL * r:BL * r + BL].transpose(1, 0, 2)  # [S, BL, I]
        sel = np.zeros((64, BL), f32)
        sel[BL * r + np.arange(BL), np.arange(BL)] = 1.0
        in_maps.append({
            "wrz": wrz.astype(BF16),
            "brz": np.ascontiguousarray(bsum[gcols].reshape(1, 256)).astype(BF16),
            "win": win.astype(BF16),
            "bin": np.ascontiguousarray(b_ih[ncols].reshape(1, 128)).astype(BF16),
            "whn": whn.astype(BF16),
            "bhn": np.ascontiguousarray(b_hh[ncols].reshape(1, 128)).astype(BF16),
            "dwsh": np.ascontiguousarray(DW[256 * r:256 * r + 256]).astype(BF16),
            "wo": _kmaj(wo, 4).astype(BF16),
            "ey": np.ascontiguousarray(
                ey.reshape(32, 128, 520).transpose(1, 0, 2)).astype(BF16),
            "wattsh": np.ascontiguousarray(
                np.asarray(W, f32)[128 * r:128 * r + 128]).astype(BF16),
            "vatt": vatt,
            "uh": uh.astype(BF16),
            "xl": xl.astype(BF16),
            "sel": sel.astype(BF16),
            "id64b": id64b,
            "id64f": id64f,
        })
    return in_maps


def _build_nc(nsteps):
    import concourse.bass as bass
    import concourse.tile as tile
    import concourse.bacc as bacc
    from concourse import mybir
    from contextlib import ExitStack

    dt = mybir.dt
    AF = mybir.ActivationFunctionType
    nc = bacc.Bacc("TRN2", target_bir_lowering=False, debug=False,
                   num_devices=NCORES)

    dI = {}
    def din(name, shape, dty=dt.bfloat16):
        dI[name] = nc.dram_tensor(name, shape, dty, kind="ExternalInput")
        return dI[name]

    din("wrz", [128, 12, 256]); din("brz", [1, 256])
    din("win", [128, 4, 128]); din("bin", [1, 128])
    din("whn", [128, 8, 128]); din("bhn", [1, 128])
    din("dwsh", [256, 1024])
    din("wo", [128, 4, KVP])
    din("ey", [128, 32, 520])
    din("wattsh", [128, 512]); din("vatt", [128, 4])
    din("uh", [128, 4, BL, S]); din("xl", [S, BL, I])
    din("sel", [64, BL])
    din("id64b", [64, 64]); din("id64f", [64, 64], dt.float32)

    op_d = nc.dram_tensor("oprobs", [nsteps, B, KV], dt.bfloat16,
                          kind="ExternalOutput")

    g1i = [nc.dram_tensor(f"g1i{t}", [512, BL], dt.bfloat16) for t in range(nsteps)]
    g1o = [nc.dram_tensor(f"g1o{t}", [4096, BL], dt.bfloat16, addr_space="Shared")
           for t in range(nsteps)]
    g2i = [nc.dram_tensor(f"g2i{t}", [128, 64], dt.bfloat16) for t in range(nsteps)]
    g2o = [nc.dram_tensor(f"g2o{t}", [1024, 64], dt.bfloat16, addr_space="Shared")
           for t in range(nsteps)]
    dwb = nc.dram_tensor("dwb", [256, 1024], dt.bfloat16)
    dwg = nc.dram_tensor("dwg", [2048, 1024], dt.bfloat16, addr_space="Shared")
    wab = nc.dram_tensor("wab", [128, 512], dt.bfloat16)
    wag = nc.dram_tensor("wag", [1024, 512], dt.bfloat16, addr_space="Shared")
    ari = [nc.dram_tensor(f"ari{t}", [64, 513], dt.float32) for t in range(nsteps)]
    aro = [nc.dram_tensor(f"aro{t}", [64, 513], dt.float32, addr_space="Shared")
           for t in range(nsteps)]
    RG = [list(range(NCORES))]

    with tile.TileContext(nc) as tc, ExitStack() as ctx:
        wp = ctx.enter_context(tc.tile_pool(name="wp", bufs=1))
        s3 = ctx.enter_context(tc.tile_pool(name="s3", bufs=3))
        s2 = ctx.enter_context(tc.tile_pool(name="s2", bufs=2))
        psm = ctx.enter_context(tc.tile_pool(name="psm", bufs=2, space="PSUM"))
        plg = ctx.enter_context(tc.tile_pool(name="plg", bufs=2, space="PSUM"))
        pye = ctx.enter_context(tc.tile_pool(name="pye", bufs=1, space="PSUM"))
        ptp = ctx.enter_context(tc.tile_pool(name="ptp", bufs=2, space="PSUM"))

        # ---- resident weights ----
        w = {}
        for name, shape, dty in [
            ("wrz", [128, 12, 256], dt.bfloat16), ("brz", [1, 256], dt.bfloat16),
            ("win", [128, 4, 128], dt.bfloat16), ("bin", [1, 128], dt.bfloat16),
            ("whn", [128, 8, 128], dt.bfloat16), ("bhn", [1, 128], dt.bfloat16),
            ("wo", [128, 4, KVP], dt.bfloat16), ("ey", [128, 32, 520], dt.bfloat16),
            ("vatt", [128, 4], dt.bfloat16),
            ("uh", [128, 4, BL, S], dt.bfloat16), ("xl", [S, BL, I], dt.bfloat16),
            ("sel", [64, BL], dt.bfloat16),
            ("id64b", [64, 64], dt.bfloat16), ("id64f", [64, 64], dt.float32),
        ]:
            w[name] = wp.tile(shape, dty, tag=name, name=name)
            nc.sync.dma_start(w[name][:], dI[name][:])

        nc.sync.dma_start(dwb[:], dI["dwsh"][:])
        nc.gpsimd.collective_compute(
            "AllGather", mybir.AluOpType.bypass, replica_groups=RG,
            ins=[dwb[:]], outs=[dwg[:]])
        w["dw"] = wp.tile([128, 16, 1024], dt.bfloat16, tag="dw", name="dw")
        nc.sync.dma_start(w["dw"][:], dwg.ap().rearrange("(kc p) n -> p kc n", p=128))
        nc.sync.dma_start(wab[:], dI["wattsh"][:])
        nc.gpsimd.collective_compute(
            "AllGather", mybir.AluOpType.bypass, replica_groups=RG,
            ins=[wab[:]], outs=[wag[:]])
        w["watt"] = wp.tile([128, 8, 512], dt.bfloat16, tag="watt", name="watt")
        nc.sync.dma_start(w["watt"][:], wag.ap().rearrange("(kc p) n -> p kc n", p=128))

        ones64 = wp.tile([64, 1], dt.bfloat16, tag="ones64")
        nc.vector.memset(ones64[:], 1.0)
        ones1r = wp.tile([1, 64], dt.bfloat16, tag="ones1r")
        nc.vector.memset(ones1r[:], 1.0)
        ones1c = wp.tile([1, 128], dt.bfloat16, tag="ones1c")
        nc.vector.memset(ones1c[:], 1.0)

        s_pl = wp.tile([64, 128], dt.float32, tag="s_pl")   # local plain s chunk
        nc.vector.memset(s_pl[:], 0.0)

        sT = [None] * (nsteps + 1)
        sT[0] = s3.tile([128, 8, 64], dt.bfloat16, tag="sT", name="sT0")
        nc.vector.memset(sT[0][:], 0.0)
        yembT = [None] * nsteps
        yT0 = s3.tile([128, 4, 64], dt.bfloat16, tag="yembT")
        nc.vector.memset(yT0[:], 0.0)
        el = [None] * nsteps

        def consume_ar(t):
            """AR(t) out -> yembT[t], and probs(t) scaled + written out."""
            yz = s2.tile([64, 513], dt.float32, tag="yz")
            nc.sync.dma_start(yz[:], aro[t][:])
            ziv = s2.tile([64, 1], dt.float32, tag="ziv")
            nc.vector.reciprocal(ziv[:], yz[:, 512:513])
            if t < nsteps - 1:
                yes = s2.tile([64, 512], dt.bfloat16, tag="yes")
                nc.vector.tensor_scalar_mul(yes[:], yz[:, :512], ziv[:])
                yembT[t] = s3.tile([128, 4, 64], dt.bfloat16, tag="yembT", name=f"yembT{t}")
                for cc in range(4):
                    yp = ptp.tile([128, 64], dt.bfloat16, tag="tp")
                    nc.tensor.transpose(yp[:], yes[:, 128 * cc:128 * cc + 128],
                                        w["id64b"][:])
                    nc.vector.tensor_copy(yembT[t][:, cc, :], yp[:])
            # Z^-1 as a row, then scale el -> probs, write out
            zr_ps = ptp.tile([1, 64], dt.float32, tag="tp")
            nc.tensor.transpose(zr_ps[:], ziv[:], w["id64f"][:])
            zrow = s2.tile([1, 64], dt.bfloat16, tag="zrow")
            nc.vector.tensor_copy(zrow[:], zr_ps[:])
            zvb_ps = psm.tile([128, 64], dt.float32, tag="mid")
            nc.tensor.matmul(zvb_ps[:], ones1c[:], zrow[:], start=True, stop=True)
            zvb = s2.tile([128, 64], dt.bfloat16, tag="zvb")
            nc.vector.tensor_copy(zvb[:], zvb_ps[:])
            pr = s2.tile([128, 64, 32], dt.bfloat16, tag="pr")
            nc.vector.tensor_mul(
                pr[:].rearrange("p b vc -> p vc b"), el[t][:],
                zvb[:].rearrange("p (o b) -> p o b", o=1).to_broadcast([128, 32, 64]))
            dst = op_d.ap()[t].rearrange("b (p vc) -> p b vc", vc=32)
            for j in range(4):
                nc.sync.dma_start(dst[:, 16 * j:16 * j + 16, :],
                                  pr[:125, 16 * j:16 * j + 16, :])

        for t in range(nsteps):
            # ---------- A: attention (local batch rows) ----------
            ws_ps = psm.tile([64, 512], dt.float32, tag="mid")
            for kc in range(8):
                nc.tensor.matmul(ws_ps[:], sT[t][:, kc, :], w["watt"][:, kc, :],
                                 start=(kc == 0), stop=(kc == 7))
            ws_sb = s2.tile([64, 512], dt.bfloat16, tag="ws_sb")
            nc.vector.tensor_copy(ws_sb[:], ws_ps[:])
            wst = s2.tile([128, 4, BL], dt.bfloat16, tag="wst")
            for cc in range(4):
                wp_ps = ptp.tile([128, BL], dt.bfloat16, tag="tp")
                nc.tensor.transpose(wp_ps[:], ws_sb[:, 128 * cc:128 * cc + 128],
                                    w["sel"][:])
                nc.vector.tensor_copy(wst[:, cc, :], wp_ps[:])
            th = s2.tile([128, 4, BL, S], dt.bfloat16, tag="th")
            for cc in range(4):
                tt = s2.tile([128, BL, S], dt.bfloat16, tag="tt")
                nc.vector.tensor_add(
                    tt[:], w["uh"][:, cc, :, :],
                    wst[:, cc, :].rearrange("p (b o) -> p b o", o=1)
                    .to_broadcast([128, BL, S]))
                nc.scalar.activation(th[:, cc, :, :], tt[:], AF.Tanh)
            e_ps = psm.tile([64, BL], dt.float32, tag="mid")
            for bb in range(BL):
                for cc in range(4):
                    nc.tensor.matmul(e_ps[:, bb:bb + 1], th[:, cc, bb, :],
                                     w["vatt"][:, cc:cc + 1],
                                     start=(cc == 0), stop=(cc == 3))
            ex = s2.tile([64, BL], dt.bfloat16, tag="ex")
            nc.scalar.activation(ex[:], e_ps[:], AF.Exp)
            zr_ps = psm.tile([1, BL], dt.float32, tag="mid")
            nc.tensor.matmul(zr_ps[:], ones64[:], ex[:], start=True, stop=True)
            zir = s2.tile([1, BL], dt.bfloat16, tag="zir")
            with nc.allow_low_precision(reason="bf16 softmax scale is fine"):
                nc.vector.reciprocal(zir[:], zr_ps[:])
            zb_ps = psm.tile([128, BL], dt.float32, tag="mid")
            nc.tensor.matmul(zb_ps[:], ones1c[:], zir[:], start=True, stop=True)
            zbc = s2.tile([128, BL], dt.float32, tag="zbc")
            nc.vector.tensor_copy(zbc[:], zb_ps[:])
            ctx_ps = psm.tile([128, 32], dt.float32, tag="mid")
            for cc in range(4):
                for bb in range(BL):
                    nc.tensor.matmul(ctx_ps[:, 8 * cc + bb:8 * cc + bb + 1],
                                     w["xl"][:, bb, 128 * cc:128 * cc + 128],
                                     ex[:, bb:bb + 1], start=True, stop=True)
            ctx_sb = s2.tile([128, 32], dt.bfloat16, tag="ctx_sb")
            nc.vector.tensor_mul(
                ctx_sb[:].rearrange("p (cc b) -> p cc b", b=BL),
                ctx_ps[:].rearrange("p (cc b) -> p cc b", b=BL),
                zbc[:].rearrange("p (o b) -> p o b", o=1).to_broadcast([128, 4, BL]))
            nc.sync.dma_start(
                g1i[t].ap().rearrange("(cc p) b -> p cc b", p=128),
                ctx_sb[:].rearrange("p (cc b) -> p cc b", b=BL))
            nc.gpsimd.collective_compute(
                "AllGather", mybir.AluOpType.bypass, replica_groups=RG,
                ins=[g1i[t][:]], outs=[g1o[t][:]])
            ctxT = s3.tile([128, 4, 64], dt.bfloat16, tag="ctxT")
            g1ov = g1o[t].ap().rearrange("(r cc p) b -> p cc r b", p=128, cc=4)
            for cc in range(4):
                nc.sync.dma_start(
                    ctxT[:, cc, :].rearrange("p (r j) -> p r j", j=BL),
                    g1ov[:, cc, :, :])

            # ---------- B: GRU (local 128 h-features) ----------
            rz_ps = psm.tile([64, 256], dt.float32, tag="mid")
            for kc in range(4):
                nc.tensor.matmul(rz_ps[:], ctxT[:, kc, :], w["wrz"][:, kc, :],
                                 start=(kc == 0), stop=False)
            for kc in range(8):
                nc.tensor.matmul(rz_ps[:], sT[t][:, kc, :], w["wrz"][:, 4 + kc, :],
                                 start=False, stop=False)
            nc.tensor.matmul(rz_ps[:], ones1r[:], w["brz"][:],
                             start=False, stop=True)
            in_ps = psm.tile([64, 128], dt.float32, tag="mid")
            for kc in range(4):
                nc.tensor.matmul(in_ps[:], ctxT[:, kc, :], w["win"][:, kc, :],
                                 start=(kc == 0), stop=False)
            nc.tensor.matmul(in_ps[:], ones1r[:], w["bin"][:],
                             start=False, stop=True)
            hn_ps = psm.tile([64, 128], dt.float32, tag="mid")
            for kc in range(8):
                nc.tensor.matmul(hn_ps[:], sT[t][:, kc, :], w["whn"][:, kc, :],
                                 start=(kc == 0), stop=False)
            nc.tensor.matmul(hn_ps[:], ones1r[:], w["bhn"][:],
                             start=False, stop=True)
            rzs = s2.tile([64, 256], dt.float32, tag="rzs")
            nc.scalar.activation(rzs[:], rz_ps[:], AF.Sigmoid)
            rh = s2.tile([64, 128], dt.float32, tag="rh")
            nc.vector.tensor_mul(rh[:], rzs[:, :128], hn_ps[:])
            npre = s2.tile([64, 128], dt.float32, tag="npre")
            nc.vector.tensor_add(npre[:], in_ps[:], rh[:])
            nn = s2.tile([64, 128], dt.float32, tag="nn")
            nc.scalar.activation(nn[:], npre[:], AF.Tanh)
            smn = s2.tile([64, 128], dt.float32, tag="smn")
            nc.vector.tensor_sub(smn[:], s_pl[:], nn[:])
            zsm = s2.tile([64, 128], dt.float32, tag="zsm")
            nc.vector.tensor_mul(zsm[:], rzs[:, 128:], smn[:])
            nc.vector.tensor_add(s_pl[:], nn[:], zsm[:])
            sn_ps = ptp.tile([128, 64], dt.float32, tag="tp")
            nc.tensor.transpose(sn_ps[:], s_pl[:], w["id64f"][:])
            snT = s2.tile([128, 64], dt.bfloat16, tag="snT")
            nc.vector.tensor_copy(snT[:], sn_ps[:])
            nc.sync.dma_start(g2i[t][:], snT[:])
            nc.gpsimd.collective_compute(
                "AllGather", mybir.AluOpType.bypass, replica_groups=RG,
                ins=[g2i[t][:]], outs=[g2o[t][:]])
            sT[t + 1] = s3.tile([128, 8, 64], dt.bfloat16, tag="sT", name=f"sT{t+1}")
            nc.sync.dma_start(sT[t + 1][:],
                              g2o[t].ap().rearrange("(kc p) b -> p kc b", p=128))

            # ---------- consume AR(t-1) -> yembT[t-1], probs(t-1) ----------
            if t > 0:
                consume_ar(t - 1)
            yprev = yT0 if t == 0 else yembT[t - 1]

            # ---------- C: deep output (replicated, full batch) ----------
            t_ps = pye.tile([64, 1024], dt.float32, tag="acc")
            for nc2 in range(2):
                o = t_ps[:, 512 * nc2:512 * nc2 + 512]
                for kc in range(16):
                    lh = (sT[t + 1][:, kc, :] if kc < 8 else
                          (yprev[:, kc - 8, :] if kc < 12 else
                           ctxT[:, kc - 12, :]))
                    nc.tensor.matmul(o, lh, w["dw"][:, kc, 512 * nc2:512 * nc2 + 512],
                                     start=(kc == 0), stop=(kc == 15))
            tm = s2.tile([64, 512], dt.bfloat16, tag="tm")
            nc.vector.reduce_max(tm[:].rearrange("p (d q) -> p d q", q=1),
                                 t_ps[:].rearrange("p (d two) -> p d two", two=2),
                                 axis=mybir.AxisListType.X)
            tmT = s2.tile([128, 4, 64], dt.bfloat16, tag="tmT")
            for cc in range(4):
                tp = ptp.tile([128, 64], dt.bfloat16, tag="tp")
                nc.tensor.transpose(tp[:], tm[:, 128 * cc:128 * cc + 128],
                                    w["id64b"][:])
                nc.vector.tensor_copy(tmT[:, cc, :], tp[:])

            # ---------- D: vocab shard ----------
            el[t] = s3.tile([128, 32, 64], dt.bfloat16, tag="el", name=f"el{t}")
            for g in range(4):
                lg = plg.tile([128, 512], dt.float32, tag="lg")
                for q in range(8):
                    vc = 8 * g + q
                    for kc in range(4):
                        nc.tensor.matmul(lg[:, 64 * q:64 * q + 64],
                                         w["wo"][:, kc, 128 * vc:128 * vc + 128],
                                         tmT[:, kc, :],
                                         start=(kc == 0), stop=(kc == 3))
                nc.scalar.activation(el[t][:, 8 * g:8 * g + 8, :], lg[:], AF.Exp)
            ye_ps = pye.tile([64, 513], dt.float32, tag="acc")
            for vc in range(32):
                nc.tensor.matmul(ye_ps[:, :512], el[t][:, vc, :],
                                 w["ey"][:, vc, :512],
                                 start=(vc == 0), stop=(vc == 31))
                nc.tensor.matmul(ye_ps[:, 512:513], el[t][:, vc, :],
                                 w["ey"][:, vc, 512:513],
                                 start=(vc == 0), stop=(vc == 31))
            ye_sb = s2.tile([64, 513], dt.float32, tag="ye_sb")
            nc.vector.tensor_copy(ye_sb[:], ye_ps[:])
            nc.sync.dma_start(ari[t][:], ye_sb[:])
            nc.gpsimd.collective_compute(
                "AllReduce", mybir.AluOpType.add, replica_groups=RG,
                ins=[ari[t][:]], outs=[aro[t][:]])

        consume_ar(nsteps - 1)

    nc.compile()
    return nc



def _upload_inputs(in_maps):
    """Start async sharded upload of all inputs; returns {name: jax.Array}."""
    import jax
    from jax.sharding import Mesh, PartitionSpec, NamedSharding
    n_cores = len(in_maps)
    devices = jax.devices()[:n_cores]
    mesh = Mesh(np.array(devices), ("core",))
    sh = NamedSharding(mesh, PartitionSpec("core"))
    out = {}
    for name in in_maps[0]:
        cat = np.concatenate([np.asarray(m[name]) for m in in_maps], axis=0)
        out[name] = jax.device_put(cat, sh)
    return out


def _run_pjrt_fast(nc, uploaded, n_cores):
    """run_bass_via_pjrt with pre-uploaded inputs, device-side zero outputs
    (skips a 268MB host upload) and threaded per-shard output fetch."""
    import jax
    import jax.numpy as jnp
    from concurrent.futures import ThreadPoolExecutor
    from jax.experimental.shard_map import shard_map
    from jax.sharding import Mesh, PartitionSpec, NamedSharding
    from concourse import mybir
    from concourse.bass2jax import (_bass_exec_p, partition_id_tensor,
                                    install_neuronx_cc_hook)

    install_neuronx_cc_hook()
    partition_name = (nc.partition_id_tensor.name
                      if nc.partition_id_tensor else None)
    in_names, out_names, out_avals = [], [], []
    for alloc in nc.m.functions[0].allocations:
        if not isinstance(alloc, mybir.MemoryLocationSet):
            continue
        name = alloc.memorylocations[0].name
        if alloc.kind == "ExternalInput":
            if name != partition_name:
                in_names.append(name)
        elif alloc.kind == "ExternalOutput":
            out_names.append(name)
            out_avals.append(jax.core.ShapedArray(tuple(alloc.tensor_shape),
                                                  mybir.dt.np(alloc.dtype)))
    n_params = len(in_names)
    n_outs = len(out_avals)
    in_names = in_names + out_names
    if partition_name is not None:
        in_names.append(partition_name)
    donate = tuple(range(n_params, n_params + n_outs))

    def _body(*args):
        operands = list(args)
        if partition_name is not None:
            operands.append(partition_id_tensor())
        return tuple(_bass_exec_p.bind(
            *operands, out_avals=tuple(out_avals), in_names=tuple(in_names),
            out_names=tuple(out_names), lowering_input_output_aliases=(),
            sim_require_finite=True, sim_require_nnan=True, nc=nc))

    devices = jax.devices()[:n_cores]
    mesh = Mesh(np.array(devices), ("core",))
    spec = PartitionSpec("core")
    in_specs = (spec,) * (n_params + n_outs)
    out_specs = (spec,) * n_outs
    sharded = jax.jit(
        shard_map(_body, mesh=mesh, in_specs=in_specs, out_specs=out_specs,
                  check_rep=False),
        donate_argnums=donate, keep_unused=True)
    import time as _time
    _t0 = _time.perf_counter()
    concat_in = [uploaded[in_names[i]] for i in range(n_params)]
    zero_fn = jax.jit(
        lambda: tuple(jnp.zeros((n_cores * a.shape[0], *a.shape[1:]), a.dtype)
                      for a in out_avals),
        out_shardings=tuple(NamedSharding(mesh, spec) for _ in out_avals))
    out_arrs = sharded(*concat_in, *zero_fn())

    import time as _time
    _td = _time.perf_counter()

    def fetch(i):
        shards = sorted(out_arrs[i].addressable_shards,
                        key=lambda s: s.index[0].start or 0)
        with ThreadPoolExecutor(max_workers=8) as ex:
            datas = list(ex.map(lambda s: np.asarray(s.data), shards))
        return datas
    fetched = [fetch(i) for i in range(n_outs)]
    print(f"[runner] dispatch={_td-_t0:.1f}s fetch={_time.perf_counter()-_td:.1f}s",
          file=sys.stderr)
    return [{name: fetched[i][c] for i, name in enumerate(out_names)}
            for c in range(n_cores)]


_CACHE = {}


def kernel(input_seq, Ey_t, W, U, b, v, W_ih, W_hh, b_ih, b_hh,
           U_o, V_o, C_o, W_o, _nsteps=S):
    import time
    from concourse import bass_utils
    t0 = time.perf_counter()
    in_maps = _build_in_maps(input_seq, Ey_t, W, U, b, v, W_ih, W_hh,
                             b_ih, b_hh, U_o, V_o, C_o, W_o)
    uploaded = _upload_inputs(in_maps)   # async; overlaps the build below
    t1 = time.perf_counter()
    if _nsteps not in _CACHE:
        _CACHE[_nsteps] = _build_nc(_nsteps)
    nc = _CACHE[_nsteps]
    t2 = time.perf_counter()
    results = _run_pjrt_fast(nc, uploaded, NCORES)
    t3 = time.perf_counter()
    out = np.empty((_nsteps, B, KY), np.float32)
    for r in range(NCORES):
        out[:, :, KV * r:KV * r + KV] = \
            results[r]["oprobs"].astype(np.float32)
    t4 = time.perf_counter()
    print(f"[kernel] host-prep={t1-t0:.1f}s build={t2-t1:.1f}s "
          f"run={t3-t2:.1f}s assemble={t4-t3:.1f}s", file=sys.stderr)
    return out


# revision 22
# speedup vs baseline: 1.0278x; 1.0278x over previous
"""Bahdanau-attention GRU decoder on 8 trn2 NeuronCores (Bass/Tile).

Sharding: vocab (32000) split 8x4000 (padded 4096) per core for the two big
GEMMs (logits = tm @ W_o, yemb = probs @ Ey); attention batch-sharded (8 rows
per core); GRU gate features sharded (128 h per core); deep-output replicated.
Per step: AllGather(ctx), AllGather(s_new^T chunk), AllReduce(yemb partial +
softmax-Z partial).  All matmul operands bf16, accumulation f32.
"""
import sys, os
sys.path.insert(0, '/opt/trn_rl_repo')

import numpy as np
import ml_dtypes

os.environ.setdefault("JAX_COMPILATION_CACHE_DIR", "/root/.jax_cache")
os.environ.setdefault("JAX_PERSISTENT_CACHE_MIN_ENTRY_SIZE_BYTES", "-1")
os.environ.setdefault("JAX_PERSISTENT_CACHE_MIN_COMPILE_TIME_SECS", "0")

BF16 = ml_dtypes.bfloat16
NCORES = 8
B, S, I, C, H, D, M, KY = 64, 64, 512, 512, 1024, 512, 512, 32000
KV = KY // NCORES          # 4000 real vocab cols per core
KVP = 4096                 # padded
BL = B // NCORES           # 8 local batch rows
# chunk vc / array-col m holds vocab col m*32+vc so the output DMA
# (partition-major) lands at linear offset p*32+vc
_PERM = (np.arange(128)[None, :] * 32 + np.arange(32)[:, None]).reshape(-1)


def _kmaj(a, nk):
    """[nk*128, N] -> [128, nk, N] with (p, kc, n) = a[kc*128+p, n]."""
    K, N = a.shape
    assert K == nk * 128
    return np.ascontiguousarray(a.reshape(nk, 128, N).transpose(1, 0, 2))


def _build_in_maps(input_seq, Ey_t, W, U, b, v, W_ih, W_hh, b_ih, b_hh,
                   U_o, V_o, C_o, W_o):
    f32 = np.float32
    input_seq = np.asarray(input_seq, f32)
    U_h = (input_seq.reshape(B * S, I).astype(f32) @ np.asarray(U, f32)
           + np.asarray(b, f32)).reshape(B, S, C)
    W_ihT = np.asarray(W_ih, f32).T     # [C, 3H]
    W_hhT = np.asarray(W_hh, f32).T     # [H, 3H]
    bsum = np.asarray(b_ih, f32) + np.asarray(b_hh, f32)
    DW = np.vstack([np.asarray(U_o, f32), np.asarray(V_o, f32),
                    np.asarray(C_o, f32)])          # [2H, H]
    vatt = np.ascontiguousarray(np.asarray(v, f32).reshape(4, 128).T).astype(BF16)
    id64b = np.eye(64, dtype=BF16)
    id64f = np.eye(64, dtype=f32)
    W_o = np.asarray(W_o, f32)
    Ey_t = np.asarray(Ey_t, f32)

    in_maps = []
    for r in range(NCORES):
        hc = slice(128 * r, 128 * r + 128)
        gcols = np.r_[128 * r:128 * r + 128, 1024 + 128 * r:1024 + 128 * r + 128]
        ncols = np.arange(2048 + 128 * r, 2048 + 128 * r + 128)
        wrz = np.concatenate([_kmaj(W_ihT[:, gcols], 4),
                              _kmaj(W_hhT[:, gcols], 8)], axis=1)
        win = _kmaj(W_ihT[:, ncols], 4)
        whn = _kmaj(W_hhT[:, ncols], 8)
        wo = np.zeros((512, KVP), f32)
        wo[:, :KV] = W_o[:, KV * r:KV * r + KV]
        ey = np.zeros((KVP, 520), f32)
        ey[:KV, :512] = Ey_t[KV * r:KV * r + KV]
        ey[:KV, 512] = 1.0
        wo = wo[:, _PERM]
        ey = ey[_PERM, :]
        uh = U_h[BL * r:BL * r + BL].transpose(2, 0, 1)       # [C, BL, S]
        uh = uh.reshape(4, 128, BL, S).transpose(1, 0, 2, 3)  # [128, 4, BL, S]
        xl = input_seq[BL * r:BL * r + BL].transpose(1, 0, 2)  # [S, BL, I]
        sel = np.zeros((64, BL), f32)
        sel[BL * r + np.arange(BL), np.arange(BL)] = 1.0
        in_maps.append({
            "wrz": wrz.astype(BF16),
            "brz": np.ascontiguousarray(bsum[gcols].reshape(1, 256)).astype(BF16),
            "win": win.astype(BF16),
            "bin": np.ascontiguousarray(b_ih[ncols].reshape(1, 128)).astype(BF16),
            "whn": whn.astype(BF16),
            "bhn": np.ascontiguousarray(b_hh[ncols].reshape(1, 128)).astype(BF16),
            "dwsh": np.ascontiguousarray(DW[256 * r:256 * r + 256]).astype(BF16),
            "wo": _kmaj(wo, 4).astype(BF16),
            "ey": np.ascontiguousarray(
                ey.reshape(32, 128, 520).transpose(1, 0, 2)).astype(BF16),
            "wattsh": np.ascontiguousarray(
                np.asarray(W, f32)[128 * r:128 * r + 128]).astype(BF16),
            "vatt": vatt,
            "uh": uh.astype(BF16),
            "xl": xl.astype(BF16),
            "sel": sel.astype(BF16),
            "id64b": id64b,
            "id64f": id64f,
        })
    return in_maps


def _build_nc(nsteps):
    import concourse.bass as bass
    import concourse.tile as tile
    import concourse.bacc as bacc
    from concourse import mybir
    from contextlib import ExitStack

    dt = mybir.dt
    AF = mybir.ActivationFunctionType
    nc = bacc.Bacc("TRN2", target_bir_lowering=False, debug=False,
                   num_devices=NCORES)

    dI = {}
    def din(name, shape, dty=dt.bfloat16):
        dI[name] = nc.dram_tensor(name, shape, dty, kind="ExternalInput")
        return dI[name]

    din("wrz", [128, 12, 256]); din("brz", [1, 256])
    din("win", [128, 4, 128]); din("bin", [1, 128])
    din("whn", [128, 8, 128]); din("bhn", [1, 128])
    din("dwsh", [256, 1024])
    din("wo", [128, 4, KVP])
    din("ey", [128, 32, 520])
    din("wattsh", [128, 512]); din("vatt", [128, 4])
    din("uh", [128, 4, BL, S]); din("xl", [S, BL, I])
    din("sel", [64, BL])
    din("id64b", [64, 64]); din("id64f", [64, 64], dt.float32)

    op_d = nc.dram_tensor("oprobs", [nsteps, B, KV], dt.bfloat16,
                          kind="ExternalOutput")

    g1i = [nc.dram_tensor(f"g1i{t}", [512, BL], dt.bfloat16) for t in range(nsteps)]
    g1o = [nc.dram_tensor(f"g1o{t}", [4096, BL], dt.bfloat16, addr_space="Shared")
           for t in range(nsteps)]
    g2i = [nc.dram_tensor(f"g2i{t}", [128, 64], dt.bfloat16) for t in range(nsteps)]
    g2o = [nc.dram_tensor(f"g2o{t}", [1024, 64], dt.bfloat16, addr_space="Shared")
           for t in range(nsteps)]
    dwb = nc.dram_tensor("dwb", [256, 1024], dt.bfloat16)
    dwg = nc.dram_tensor("dwg", [2048, 1024], dt.bfloat16, addr_space="Shared")
    wab = nc.dram_tensor("wab", [128, 512], dt.bfloat16)
    wag = nc.dram_tensor("wag", [1024, 512], dt.bfloat16, addr_space="Shared")
    ari = [nc.dram_tensor(f"ari{t}", [64, 513], dt.float32) for t in range(nsteps)]
    aro = [nc.dram_tensor(f"aro{t}", [64, 513], dt.float32, addr_space="Shared")
           for t in range(nsteps)]
    RG = [list(range(NCORES))]

    with tile.TileContext(nc) as tc, ExitStack() as ctx:
        wp = ctx.enter_context(tc.tile_pool(name="wp", bufs=1))
        s3 = ctx.enter_context(tc.tile_pool(name="s3", bufs=3))
        s2 = ctx.enter_context(tc.tile_pool(name="s2", bufs=2))
        psm = ctx.enter_context(tc.tile_pool(name="psm", bufs=2, space="PSUM"))
        plg = ctx.enter_context(tc.tile_pool(name="plg", bufs=2, space="PSUM"))
        pye = ctx.enter_context(tc.tile_pool(name="pye", bufs=1, space="PSUM"))
        ptp = ctx.enter_context(tc.tile_pool(name="ptp", bufs=2, space="PSUM"))

        # ---- resident weights ----
        w = {}
        for name, shape, dty in [
            ("wrz", [128, 12, 256], dt.bfloat16), ("brz", [1, 256], dt.bfloat16),
            ("win", [128, 4, 128], dt.bfloat16), ("bin", [1, 128], dt.bfloat16),
            ("whn", [128, 8, 128], dt.bfloat16), ("bhn", [1, 128], dt.bfloat16),
            ("wo", [128, 4, KVP], dt.bfloat16), ("ey", [128, 32, 520], dt.bfloat16),
            ("vatt", [128, 4], dt.bfloat16),
            ("uh", [128, 4, BL, S], dt.bfloat16), ("xl", [S, BL, I], dt.bfloat16),
            ("sel", [64, BL], dt.bfloat16),
            ("id64b", [64, 64], dt.bfloat16), ("id64f", [64, 64], dt.float32),
        ]:
            w[name] = wp.tile(shape, dty, tag=name, name=name)
            nc.sync.dma_start(w[name][:], dI[name][:])

        nc.sync.dma_start(dwb[:], dI["dwsh"][:])
        nc.gpsimd.collective_compute(
            "AllGather", mybir.AluOpType.bypass, replica_groups=RG,
            ins=[dwb[:]], outs=[dwg[:]])
        w["dw"] = wp.tile([128, 16, 1024], dt.bfloat16, tag="dw", name="dw")
        nc.sync.dma_start(w["dw"][:], dwg.ap().rearrange("(kc p) n -> p kc n", p=128))
        nc.sync.dma_start(wab[:], dI["wattsh"][:])
        nc.gpsimd.collective_compute(
            "AllGather", mybir.AluOpType.bypass, replica_groups=RG,
            ins=[wab[:]], outs=[wag[:]])
        w["watt"] = wp.tile([128, 8, 512], dt.bfloat16, tag="watt", name="watt")
        nc.sync.dma_start(w["watt"][:], wag.ap().rearrange("(kc p) n -> p kc n", p=128))

        ones64 = wp.tile([64, 1], dt.bfloat16, tag="ones64")
        nc.vector.memset(ones64[:], 1.0)
        ones1r = wp.tile([1, 64], dt.bfloat16, tag="ones1r")
        nc.vector.memset(ones1r[:], 1.0)
        ones1c = wp.tile([1, 128], dt.bfloat16, tag="ones1c")
        nc.vector.memset(ones1c[:], 1.0)

        s_pl = wp.tile([64, 128], dt.float32, tag="s_pl")   # local plain s chunk
        nc.vector.memset(s_pl[:], 0.0)

        sT = [None] * (nsteps + 1)
        sT[0] = s3.tile([128, 8, 64], dt.bfloat16, tag="sT", name="sT0")
        nc.vector.memset(sT[0][:], 0.0)
        yembT = [None] * nsteps
        yT0 = s3.tile([128, 4, 64], dt.bfloat16, tag="yembT")
        nc.vector.memset(yT0[:], 0.0)
        el = [None] * nsteps

        def consume_ar(t):
            """AR(t) out -> yembT[t], and probs(t) scaled + written out."""
            yz = s2.tile([64, 513], dt.float32, tag="yz")
            nc.sync.dma_start(yz[:], aro[t][:])
            ziv = s2.tile([64, 1], dt.float32, tag="ziv")
            nc.vector.reciprocal(ziv[:], yz[:, 512:513])
            if t < nsteps - 1:
                yes = s2.tile([64, 512], dt.bfloat16, tag="yes")
                nc.vector.tensor_scalar_mul(yes[:], yz[:, :512], ziv[:])
                yembT[t] = s3.tile([128, 4, 64], dt.bfloat16, tag="yembT", name=f"yembT{t}")
                for cc in range(4):
                    yp = ptp.tile([128, 64], dt.bfloat16, tag="tp")
                    nc.tensor.transpose(yp[:], yes[:, 128 * cc:128 * cc + 128],
                                        w["id64b"][:])
                    nc.vector.tensor_copy(yembT[t][:, cc, :], yp[:])
            # Z^-1 as a row, then scale el -> probs, write out
            zr_ps = ptp.tile([1, 64], dt.float32, tag="tp")
            nc.tensor.transpose(zr_ps[:], ziv[:], w["id64f"][:])
            zrow = s2.tile([1, 64], dt.bfloat16, tag="zrow")
            nc.vector.tensor_copy(zrow[:], zr_ps[:])
            zvb_ps = psm.tile([128, 64], dt.float32, tag="mid")
            nc.tensor.matmul(zvb_ps[:], ones1c[:], zrow[:], start=True, stop=True)
            zvb = s2.tile([128, 64], dt.bfloat16, tag="zvb")
            nc.vector.tensor_copy(zvb[:], zvb_ps[:])
            pr = s2.tile([128, 64, 32], dt.bfloat16, tag="pr")
            nc.vector.tensor_mul(
                pr[:].rearrange("p b vc -> p vc b"), el[t][:],
                zvb[:].rearrange("p (o b) -> p o b", o=1).to_broadcast([128, 32, 64]))
            dst = op_d.ap()[t].rearrange("b (p vc) -> p b vc", vc=32)
            for j in range(4):
                nc.sync.dma_start(dst[:, 16 * j:16 * j + 16, :],
                                  pr[:125, 16 * j:16 * j + 16, :])

        for t in range(nsteps):
            # ---------- A: attention (local batch rows) ----------
            ws_ps = psm.tile([64, 512], dt.float32, tag="mid")
            for kc in range(8):
                nc.tensor.matmul(ws_ps[:], sT[t][:, kc, :], w["watt"][:, kc, :],
                                 start=(kc == 0), stop=(kc == 7))
            ws_sb = s2.tile([64, 512], dt.bfloat16, tag="ws_sb")
            nc.vector.tensor_copy(ws_sb[:], ws_ps[:])
            wst = s2.tile([128, 4, BL], dt.bfloat16, tag="wst")
            for cc in range(4):
                wp_ps = ptp.tile([128, BL], dt.bfloat16, tag="tp")
                nc.tensor.transpose(wp_ps[:], ws_sb[:, 128 * cc:128 * cc + 128],
                                    w["sel"][:])
                nc.vector.tensor_copy(wst[:, cc, :], wp_ps[:])
            th = s2.tile([128, 4, BL, S], dt.bfloat16, tag="th")
            for cc in range(4):
                tt = s2.tile([128, BL, S], dt.bfloat16, tag="tt")
                nc.vector.tensor_add(
                    tt[:], w["uh"][:, cc, :, :],
                    wst[:, cc, :].rearrange("p (b o) -> p b o", o=1)
                    .to_broadcast([128, BL, S]))
                nc.scalar.activation(th[:, cc, :, :], tt[:], AF.Tanh)
            e_ps = psm.tile([64, BL], dt.float32, tag="mid")
            for bb in range(BL):
                for cc in range(4):
                    nc.tensor.matmul(e_ps[:, bb:bb + 1], th[:, cc, bb, :],
                                     w["vatt"][:, cc:cc + 1],
                                     start=(cc == 0), stop=(cc == 3))
            ex = s2.tile([64, BL], dt.bfloat16, tag="ex")
            nc.scalar.activation(ex[:], e_ps[:], AF.Exp)
            zr_ps = psm.tile([1, BL], dt.float32, tag="mid")
            nc.tensor.matmul(zr_ps[:], ones64[:], ex[:], start=True, stop=True)
            zir = s2.tile([1, BL], dt.bfloat16, tag="zir")
            with nc.allow_low_precision(reason="bf16 softmax scale is fine"):
                nc.vector.reciprocal(zir[:], zr_ps[:])
            zb_ps = psm.tile([128, BL], dt.float32, tag="mid")
            nc.tensor.matmul(zb_ps[:], ones1c[:], zir[:], start=True, stop=True)
            zbc = s2.tile([128, BL], dt.float32, tag="zbc")
            nc.vector.tensor_copy(zbc[:], zb_ps[:])
            ctx_ps = psm.tile([128, 32], dt.float32, tag="mid")
            for cc in range(4):
                for bb in range(BL):
                    nc.tensor.matmul(ctx_ps[:, 8 * cc + bb:8 * cc + bb + 1],
                                     w["xl"][:, bb, 128 * cc:128 * cc + 128],
                                     ex[:, bb:bb + 1], start=True, stop=True)
            ctx_sb = s2.tile([128, 32], dt.bfloat16, tag="ctx_sb")
            nc.vector.tensor_mul(
                ctx_sb[:].rearrange("p (cc b) -> p cc b", b=BL),
                ctx_ps[:].rearrange("p (cc b) -> p cc b", b=BL),
                zbc[:].rearrange("p (o b) -> p o b", o=1).to_broadcast([128, 4, BL]))
            nc.sync.dma_start(
                g1i[t].ap().rearrange("(cc p) b -> p cc b", p=128),
                ctx_sb[:].rearrange("p (cc b) -> p cc b", b=BL))
            nc.gpsimd.collective_compute(
                "AllGather", mybir.AluOpType.bypass, replica_groups=RG,
                ins=[g1i[t][:]], outs=[g1o[t][:]])
            ctxT = s3.tile([128, 4, 64], dt.bfloat16, tag="ctxT")
            g1ov = g1o[t].ap().rearrange("(r cc p) b -> p cc r b", p=128, cc=4)
            for cc in range(4):
                nc.sync.dma_start(
                    ctxT[:, cc, :].rearrange("p (r j) -> p r j", j=BL),
                    g1ov[:, cc, :, :])

            # ---------- B: GRU (local 128 h-features) ----------
            rz_ps = psm.tile([64, 256], dt.float32, tag="mid")
            for kc in range(4):
                nc.tensor.matmul(rz_ps[:], ctxT[:, kc, :], w["wrz"][:, kc, :],
                                 start=(kc == 0), stop=False)
            for kc in range(8):
                nc.tensor.matmul(rz_ps[:], sT[t][:, kc, :], w["wrz"][:, 4 + kc, :],
                                 start=False, stop=False)
            nc.tensor.matmul(rz_ps[:], ones1r[:], w["brz"][:],
                             start=False, stop=True)
            in_ps = psm.tile([64, 128], dt.float32, tag="mid")
            for kc in range(4):
                nc.tensor.matmul(in_ps[:], ctxT[:, kc, :], w["win"][:, kc, :],
                                 start=(kc == 0), stop=False)
            nc.tensor.matmul(in_ps[:], ones1r[:], w["bin"][:],
                             start=False, stop=True)
            hn_ps = psm.tile([64, 128], dt.float32, tag="mid")
            for kc in range(8):
                nc.tensor.matmul(hn_ps[:], sT[t][:, kc, :], w["whn"][:, kc, :],
                                 start=(kc == 0), stop=False)
            nc.tensor.matmul(hn_ps[:], ones1r[:], w["bhn"][:],
                             start=False, stop=True)
            rzs = s2.tile([64, 256], dt.float32, tag="rzs")
            nc.scalar.activation(rzs[:], rz_ps[:], AF.Sigmoid)
            rh = s2.tile([64, 128], dt.float32, tag="rh")
            nc.vector.tensor_mul(rh[:], rzs[:, :128], hn_ps[:])
            npre = s2.tile([64, 128], dt.float32, tag="npre")
            nc.vector.tensor_add(npre[:], in_ps[:], rh[:])
            nn = s2.tile([64, 128], dt.float32, tag="nn")
            nc.scalar.activation(nn[:], npre[:], AF.Tanh)
            smn = s2.tile([64, 128], dt.float32, tag="smn")
            nc.vector.tensor_sub(smn[:], s_pl[:], nn[:])
            zsm = s2.tile([64, 128], dt.float32, tag="zsm")
            nc.vector.tensor_mul(zsm[:], rzs[:, 128:], smn[:])
            nc.vector.tensor_add(s_pl[:], nn[:], zsm[:])
            sn_ps = ptp.tile([128, 64], dt.float32, tag="tp")
            nc.tensor.transpose(sn_ps[:], s_pl[:], w["id64f"][:])
            snT = s2.tile([128, 64], dt.bfloat16, tag="snT")
            nc.vector.tensor_copy(snT[:], sn_ps[:])
            nc.sync.dma_start(g2i[t][:], snT[:])
            nc.gpsimd.collective_compute(
                "AllGather", mybir.AluOpType.bypass, replica_groups=RG,
                ins=[g2i[t][:]], outs=[g2o[t][:]])
            sT[t + 1] = s3.tile([128, 8, 64], dt.bfloat16, tag="sT", name=f"sT{t+1}")
            nc.sync.dma_start(sT[t + 1][:],
                              g2o[t].ap().rearrange("(kc p) b -> p kc b", p=128))

            # ---------- consume AR(t-1) -> yembT[t-1], probs(t-1) ----------
            if t > 0:
                consume_ar(t - 1)
            yprev = yT0 if t == 0 else yembT[t - 1]

            # ---------- C: deep output (replicated, full batch) ----------
            t_ps = pye.tile([64, 1024], dt.float32, tag="acc")
            for nc2 in range(2):
                o = t_ps[:, 512 * nc2:512 * nc2 + 512]
                for kc in range(16):
                    lh = (sT[t + 1][:, kc, :] if kc < 8 else
                          (yprev[:, kc - 8, :] if kc < 12 else
                           ctxT[:, kc - 12, :]))
                    nc.tensor.matmul(o, lh, w["dw"][:, kc, 512 * nc2:512 * nc2 + 512],
                                     start=(kc == 0), stop=(kc == 15))
            tm = s2.tile([64, 512], dt.bfloat16, tag="tm")
            nc.vector.reduce_max(tm[:].rearrange("p (d q) -> p d q", q=1),
                                 t_ps[:].rearrange("p (d two) -> p d two", two=2),
                                 axis=mybir.AxisListType.X)
            tmT = s2.tile([128, 4, 64], dt.bfloat16, tag="tmT")
            for cc in range(4):
                tp = ptp.tile([128, 64], dt.bfloat16, tag="tp")
                nc.tensor.transpose(tp[:], tm[:, 128 * cc:128 * cc + 128],
                                    w["id64b"][:])
                nc.vector.tensor_copy(tmT[:, cc, :], tp[:])

            # ---------- D: vocab shard ----------
            el[t] = s3.tile([128, 32, 64], dt.bfloat16, tag="el", name=f"el{t}")
            for g in range(4):
                lg = plg.tile([128, 512], dt.float32, tag="lg")
                for q in range(8):
                    vc = 8 * g + q
                    for kc in range(4):
                        nc.tensor.matmul(lg[:, 64 * q:64 * q + 64],
                                         w["wo"][:, kc, 128 * vc:128 * vc + 128],
                                         tmT[:, kc, :],
                                         start=(kc == 0), stop=(kc == 3))
                nc.scalar.activation(el[t][:, 8 * g:8 * g + 8, :], lg[:], AF.Exp)
            ye_ps = pye.tile([64, 513], dt.float32, tag="acc")
            for vc in range(32):
                nc.tensor.matmul(ye_ps[:, :512], el[t][:, vc, :],
                                 w["ey"][:, vc, :512],
                                 start=(vc == 0), stop=(vc == 31))
                nc.tensor.matmul(ye_ps[:, 512:513], el[t][:, vc, :],
                                 w["ey"][:, vc, 512:513],
                                 start=(vc == 0), stop=(vc == 31))
            ye_sb = s2.tile([64, 513], dt.float32, tag="ye_sb")
            nc.vector.tensor_copy(ye_sb[:], ye_ps[:])
            nc.sync.dma_start(ari[t][:], ye_sb[:])
            nc.gpsimd.collective_compute(
                "AllReduce", mybir.AluOpType.add, replica_groups=RG,
                ins=[ari[t][:]], outs=[aro[t][:]])

        consume_ar(nsteps - 1)

    nc.compile()
    return nc



def _upload_inputs(in_maps):
    """Start async sharded upload of all inputs; returns {name: jax.Array}."""
    import jax
    from jax.sharding import Mesh, PartitionSpec, NamedSharding
    n_cores = len(in_maps)
    devices = jax.devices()[:n_cores]
    mesh = Mesh(np.array(devices), ("core",))
    sh = NamedSharding(mesh, PartitionSpec("core"))
    out = {}
    for name in in_maps[0]:
        cat = np.concatenate([np.asarray(m[name]) for m in in_maps], axis=0)
        out[name] = jax.device_put(cat, sh)
    return out


def _run_pjrt_fast(nc, uploaded, n_cores, assemble=None):
    """run_bass_via_pjrt with pre-uploaded inputs, device-side zero outputs
    (skips a 268MB host upload) and threaded per-shard output fetch."""
    import jax
    import jax.numpy as jnp
    from concurrent.futures import ThreadPoolExecutor
    from jax.experimental.shard_map import shard_map
    from jax.sharding import Mesh, PartitionSpec, NamedSharding
    from concourse import mybir
    from concourse.bass2jax import (_bass_exec_p, partition_id_tensor,
                                    install_neuronx_cc_hook)

    install_neuronx_cc_hook()
    partition_name = (nc.partition_id_tensor.name
                      if nc.partition_id_tensor else None)
    in_names, out_names, out_avals = [], [], []
    for alloc in nc.m.functions[0].allocations:
        if not isinstance(alloc, mybir.MemoryLocationSet):
            continue
        name = alloc.memorylocations[0].name
        if alloc.kind == "ExternalInput":
            if name != partition_name:
                in_names.append(name)
        elif alloc.kind == "ExternalOutput":
            out_names.append(name)
            out_avals.append(jax.core.ShapedArray(tuple(alloc.tensor_shape),
                                                  mybir.dt.np(alloc.dtype)))
    n_params = len(in_names)
    n_outs = len(out_avals)
    in_names = in_names + out_names
    if partition_name is not None:
        in_names.append(partition_name)
    donate = tuple(range(n_params, n_params + n_outs))

    def _body(*args):
        operands = list(args)
        if partition_name is not None:
            operands.append(partition_id_tensor())
        return tuple(_bass_exec_p.bind(
            *operands, out_avals=tuple(out_avals), in_names=tuple(in_names),
            out_names=tuple(out_names), lowering_input_output_aliases=(),
            sim_require_finite=True, sim_require_nnan=True, nc=nc))

    devices = jax.devices()[:n_cores]
    mesh = Mesh(np.array(devices), ("core",))
    spec = PartitionSpec("core")
    in_specs = (spec,) * (n_params + n_outs)
    out_specs = (spec,) * n_outs
    sharded = jax.jit(
        shard_map(_body, mesh=mesh, in_specs=in_specs, out_specs=out_specs,
                  check_rep=False),
        donate_argnums=donate, keep_unused=True)
    import time as _time
    _t0 = _time.perf_counter()
    concat_in = [uploaded[in_names[i]] for i in range(n_params)]
    zero_fn = jax.jit(
        lambda: tuple(jnp.zeros((n_cores * a.shape[0], *a.shape[1:]), a.dtype)
                      for a in out_avals),
        out_shardings=tuple(NamedSharding(mesh, spec) for _ in out_avals))
    out_arrs = sharded(*concat_in, *zero_fn())

    import time as _time
    _td = _time.perf_counter()

    shards = sorted(out_arrs[0].addressable_shards,
                    key=lambda s: s.index[0].start or 0)

    def fetch_one(c):
        data = np.asarray(shards[c].data)
        if assemble is not None:
            assemble(c, data)
            return None
        return data
    with ThreadPoolExecutor(max_workers=8) as ex:
        datas = list(ex.map(fetch_one, range(n_cores)))
    print(f"[runner] dispatch={_td-_t0:.1f}s fetch={_time.perf_counter()-_td:.1f}s",
          file=sys.stderr)
    if assemble is not None:
        return None
    return [{out_names[0]: datas[c]} for c in range(n_cores)]


_CACHE = {}


def kernel(input_seq, Ey_t, W, U, b, v, W_ih, W_hh, b_ih, b_hh,
           U_o, V_o, C_o, W_o, _nsteps=S):
    import time
    from concourse import bass_utils
    t0 = time.perf_counter()
    in_maps = _build_in_maps(input_seq, Ey_t, W, U, b, v, W_ih, W_hh,
                             b_ih, b_hh, U_o, V_o, C_o, W_o)
    uploaded = _upload_inputs(in_maps)   # async; overlaps the build below
    t1 = time.perf_counter()
    if _nsteps not in _CACHE:
        _CACHE[_nsteps] = _build_nc(_nsteps)
    nc = _CACHE[_nsteps]
    t2 = time.perf_counter()
    out = np.empty((_nsteps, B, KY), np.float32)

    def _assemble(r, shard):
        out[:, :, KV * r:KV * r + KV] = shard.astype(np.float32)

    _run_pjrt_fast(nc, uploaded, NCORES, assemble=_assemble)
    t3 = time.perf_counter()
    t4 = time.perf_counter()
    print(f"[kernel] host-prep={t1-t0:.1f}s build={t2-t1:.1f}s "
          f"run={t3-t2:.1f}s assemble={t4-t3:.1f}s", file=sys.stderr)
    return out
